# revision 12
# baseline (speedup 1.0000x reference)
"""Causal multi-head attention (B=2, H=12, T=2048, D=64) on 8 Trainium2 NeuronCores.

Sharding: the 24 (batch, head) pairs are split 3-per-core across 8 cores.
Per head the device kernel computes, in transposed-score layout:

    S^T[kv, q] = K @ Q^T            (PE, fp16 in / fp32 PSUM out)
    P^T        = exp(S^T * 1/8)     (ACT, reads PSUM, writes fp16 SBUF;
                                     no max-subtraction: |scores| <= ~6)
    diag block masked by upper-triangular 0/1 matrix (DVE)
    O'^T[d+1, q] = [V | 1]^T-style ones-augmented PV matmul (PE) so that
                   row 64 of O'^T accumulates the softmax denominators
    O = (O'^T[0:64] / O'^T[64]).T   (PE transpose + DVE recip/scale)

Self-contained: only imports numpy + the installed concourse/bass stack.
"""

import os
import numpy as np

B, H, T, D = 2, 12, 2048, 64
NCORES = 8
HPC = (B * H) // NCORES      # heads per core = 3
NQT = T // 128               # 16 q tiles of 128 rows
MEGA_BLKS = 8                # kv blocks per PSUM score tile (8*128 cols = 2 banks)
SCALE = 1.0 / 8.0            # 1/sqrt(D)

_cache = {}


def build_program(
    mega_blks=MEGA_BLKS,
    smega_bufs=3,
    mask_engine="gpsimd",
    pt_bufs=3,
    oq_bufs=1,
    otp_bufs=1,
    layout="linear",
    tail_mode="pe",
    ablate=(),
):
    import concourse.bacc as bacc
    import concourse.mybir as mybir
    import concourse.tile as tile
    from concourse.masks import make_upper_triangular, make_identity

    f16 = mybir.dt.float16
    f32 = mybir.dt.float32
    Exp = mybir.ActivationFunctionType.Exp

    nc = bacc.Bacc(None)
    qT_d = nc.dram_tensor("qT", [HPC, D, T], f16, kind="ExternalInput")
    kT_d = nc.dram_tensor("kT", [HPC, D, T], f16, kind="ExternalInput")
    v_d = nc.dram_tensor("v", [HPC, T, D], f16, kind="ExternalInput")
    o_d = nc.dram_tensor("out", [HPC, T, D], f32, kind="ExternalOutput")

    with tile.TileContext(nc) as tc:
        with (
            tc.tile_pool(name="consts", bufs=1) as consts,
            tc.tile_pool(name="qk", bufs=2) as qk,
            tc.tile_pool(name="vpool", bufs=2) as vpool,
            tc.tile_pool(name="ptpool", bufs=pt_bufs) as ptpool,
            tc.tile_pool(name="odrain", bufs=2) as odrain,
            tc.tile_pool(name="otsb", bufs=2) as otsb,
            tc.tile_pool(name="rp", bufs=2) as rp,
            tc.tile_pool(name="onorm", bufs=2) as onorm,
            tc.tile_pool(name="smega", bufs=smega_bufs, space="PSUM") as smega,
            tc.tile_pool(name="oqp", bufs=oq_bufs, space="PSUM") as oqp,
            tc.tile_pool(name="otp", bufs=max(otp_bufs, 1), space="PSUM") as otp,
        ):
            U = consts.tile([128, 128], f16)
            make_upper_triangular(nc, U[:], val=1.0, diag=True)
            ident = consts.tile([128, 128], f32)
            make_identity(nc, ident[:])
            # Warm the ACT exp table set while the first DMAs are in flight.
            warm = consts.tile([128, 1], f32)
            nc.scalar.activation(warm[:], U[:, 0:1], Exp)

            for h in range(HPC):
                qt = qk.tile([D, T], f16, tag="qt")
                nc.sync.dma_start(qt[:], qT_d[h])
                kt = qk.tile([D, T], f16, tag="kt")
                nc.sync.dma_start(kt[:], kT_d[h])
                vp = vpool.tile([128, NQT, D + 1], f16)
                nc.sync.dma_start(
                    vp[:, :, 0:D], v_d[h].rearrange("(j p) d -> p j d", p=128)
                )
                nc.vector.memset(vp[:, :, D], 1.0)

                if layout == "linear":
                    blocks = [(i, j) for i in range(NQT) for j in range(i + 1)]
                    chunks = [
                        blocks[c0 : c0 + mega_blks]
                        for c0 in range(0, len(blocks), mega_blks)
                    ]
                    off = {}
                    o = 0
                    for bl in blocks:
                        off[bl] = o
                        o += 128
                    # q-tile i is fully covered once block (i, i) is emitted
                    fin_chunk = {}
                    for ci, ch in enumerate(chunks):
                        for (i, j) in ch:
                            if i == j:
                                fin_chunk.setdefault(ci, []).append(i)
                    pt = ptpool.tile([128, len(blocks) * 128], f16, tag="pt")

                    def emit_chunk(ci):
                        ch = chunks[ci]
                        sm = smega.tile([128, len(ch) * 128], f32, tag="sm")
                        for idx, (i, j) in enumerate(ch):
                            nc.tensor.matmul(
                                sm[:, idx * 128:(idx + 1) * 128],
                                kt[:, j * 128:(j + 1) * 128],
                                qt[:, i * 128:(i + 1) * 128],
                            )
                        nc.scalar.activation(
                            pt[:, off[ch[0]] : off[ch[-1]] + 128],
                            sm[:],
                            Exp,
                            scale=SCALE,
                        )

                    def emit_tail_l(oq_tile, i0):
                        if "tail" in ablate:
                            return
                        if tail_mode == "dma":
                            # drain to fp16, transpose via DMA xbar, normalize
                            od = odrain.tile([D + 1, 512], f16, tag="od16")
                            nc.vector.tensor_copy(od[:], oq_tile[:])
                            ots = otsb.tile([128, 4, D + 1], f16, tag="ots16")
                            for t in range(4):
                                nc.sync.dma_start(
                                    ots[:, t, :],
                                    od[:, t * 128:(t + 1) * 128],
                                    transpose=True,
                                )
                            r = rp.tile([128, 4], f32)
                            nc.vector.reciprocal(r[:], ots[:, :, D])
                            on = onorm.tile([128, 4, D], f32)
                            for t in range(4):
                                nc.vector.tensor_scalar_mul(
                                    on[:, t, :], ots[:, t, 0:D], r[:, t : t + 1]
                                )
                            nc.sync.dma_start(
                                o_d[h, i0 * 128:(i0 + 4) * 128, :].rearrange(
                                    "(t p) d -> p t d", p=128
                                ),
                                on[:],
                            )
                            return
                        od = odrain.tile([D + 1, 512], f32)
                        nc.vector.tensor_copy(od[:], oq_tile[:])
                        if tail_mode == "smega":
                            ot = smega.tile([128, 4, D + 1], f32, tag="sm")
                        else:
                            ot = otp.tile([128, 4, D + 1], f32)
                        for t in range(4):
                            nc.tensor.transpose(
                                ot[:, t, :],
                                od[:, t * 128:(t + 1) * 128],
                                ident[0 : D + 1, 0 : D + 1],
                            )
                        ots = otsb.tile([128, 4, D + 1], f32)
                        nc.vector.tensor_copy(ots[:], ot[:])
                        r = rp.tile([128, 4], f32)
                        nc.vector.reciprocal(r[:], ots[:, :, D])
                        on = onorm.tile([128, 4, D], f32)
                        for t in range(4):
                            nc.vector.tensor_scalar_mul(
                                on[:, t, :], ots[:, t, 0:D], r[:, t : t + 1]
                            )
                        nc.sync.dma_start(
                            o_d[h, i0 * 128:(i0 + 4) * 128, :].rearrange(
                                "(t p) d -> p t d", p=128
                            ),
                            on[:],
                        )

                    oq = None
                    pend = None

                    def emit_pv(i):
                        nonlocal oq, pend
                        nb = i + 1
                        if i % 4 == 0:
                            if pend is not None:
                                emit_tail_l(*pend)
                                pend = None
                            oq = oqp.tile([D + 1, 512], f32)
                        pd = pt[:, off[(i, i)] : off[(i, i)] + 128]
                        if "mask" not in ablate:
                            nc.gpsimd.affine_select(
                                out=pd, in_=pd,
                                compare_op=mybir.AluOpType.is_ge,
                                fill=0.0, base=0,
                                pattern=[[1, 128]], channel_multiplier=-1,
                            )
                        osl = oq[:, (i % 4) * 128:(i % 4 + 1) * 128]
                        for j in range(nb):
                            nc.tensor.matmul(
                                osl,
                                vp[:, j, :],
                                pt[:, off[(i, j)] : off[(i, j)] + 128],
                                start=(j == 0),
                                stop=(j == nb - 1),
                            )
                        if i % 4 == 3:
                            pend = (oq, i - 3)

                    emit_chunk(0)
                    for ci in range(len(chunks)):
                        if ci + 1 < len(chunks):
                            emit_chunk(ci + 1)
                        for i in fin_chunk.get(ci, []):
                            emit_pv(i)
                    emit_tail_l(*pend)
                    continue

                pts = {}

                def emit_scores(i):
                    """QK^T matmuls for q-tile i into PSUM megas + exp into pt."""
                    nb = i + 1
                    qs = qt[:, i * 128:(i + 1) * 128]
                    pt = ptpool.tile([128, nb * 128], f16, tag="pt")
                    pts[i] = pt
                    for c0 in range(0, nb, mega_blks):
                        c1 = min(c0 + mega_blks, nb)
                        sm = smega.tile([128, (c1 - c0) * 128], f32, tag="sm")
                        for j in range(c0, c1):
                            nc.tensor.matmul(
                                sm[:, (j - c0) * 128:(j - c0 + 1) * 128],
                                kt[:, j * 128:(j + 1) * 128],
                                qs,
                            )
                        if "exp" not in ablate:
                            nc.scalar.activation(
                                pt[:, c0 * 128:c1 * 128], sm[:], Exp, scale=SCALE
                            )
                        else:
                            nc.vector.tensor_copy(pt[:, c0 * 128:c1 * 128], sm[:])

                def emit_tail(oq_tile, i0):
                    """Drain + transpose + normalize + store one output quad."""
                    if "tail" in ablate:
                        return
                    od = odrain.tile([D + 1, 512], f32)
                    nc.vector.tensor_copy(od[:], oq_tile[:])
                    ot = otp.tile([128, 4, D + 1], f32)
                    for t in range(4):
                        nc.tensor.transpose(
                            ot[:, t, :],
                            od[:, t * 128:(t + 1) * 128],
                            ident[0 : D + 1, 0 : D + 1],
                        )
                    ots = otsb.tile([128, 4, D + 1], f32)
                    nc.vector.tensor_copy(ots[:], ot[:])
                    r = rp.tile([128, 4], f32)
                    nc.vector.reciprocal(r[:], ots[:, :, D])
                    on = onorm.tile([128, 4, D], f32)
                    for t in range(4):
                        nc.vector.tensor_scalar_mul(
                            on[:, t, :], ots[:, t, 0:D], r[:, t : t + 1]
                        )
                    nc.sync.dma_start(
                        o_d[h, i0 * 128:(i0 + 4) * 128, :].rearrange(
                            "(t p) d -> p t d", p=128
                        ),
                        on[:],
                    )

                emit_scores(0)
                oq = None
                pend = None
                for i in range(NQT):
                    nb = i + 1
                    if i + 1 < NQT:
                        emit_scores(i + 1)
                    if i % 4 == 0:
                        if pend is not None:
                            emit_tail(*pend)
                            pend = None
                        oq = oqp.tile([D + 1, 512], f32)
                    pt = pts[i]
                    # causal mask on the diagonal kv block: zero entries with
                    # q < kv (strictly-lower triangle of the transposed block)
                    pt_diag = pt[:, (nb - 1) * 128 : nb * 128]
                    if "mask" in ablate:
                        pass
                    elif mask_engine == "gpsimd":
                        nc.gpsimd.affine_select(
                            out=pt_diag,
                            in_=pt_diag,
                            compare_op=mybir.AluOpType.is_ge,
                            fill=0.0,
                            base=0,
                            # keep where (y - x) >= 0, i.e. col >= row
                            pattern=[[1, 128]],
                            channel_multiplier=-1,
                        )
                    else:
                        nc.vector.tensor_mul(pt_diag, pt_diag, U[:])
                    # PV: accumulate [V | ones]^T @ P^T into the output quad
                    oslice = oq[:, (i % 4) * 128:(i % 4 + 1) * 128]
                    for j in range(nb if "pv" not in ablate else 1):
                        nc.tensor.matmul(
                            oslice,
                            vp[:, j, :],
                            pt[:, j * 128:(j + 1) * 128],
                            start=(j == 0),
                            stop=(j == nb - 1),
                        )
                    if i % 4 == 3:
                        pend = (oq, i - 3)
                    del pts[i]
                emit_tail(*pend)

    nc.compile()
    return nc


def _get_program():
    if "nc" not in _cache:
        os.environ.setdefault("MYCRO_LOCAL_CACHE", "1")
        _cache["nc"] = build_program()
    return _cache["nc"]


def kernel(q, k, v):
    from concourse.bass_utils import run_bass_kernel_spmd

    q = np.asarray(q).reshape(B * H, T, D).astype(np.float16)
    k = np.asarray(k).reshape(B * H, T, D).astype(np.float16)
    v = np.ascontiguousarray(np.asarray(v).reshape(B * H, T, D).astype(np.float16))
    qT = np.ascontiguousarray(q.transpose(0, 2, 1))
    kT = np.ascontiguousarray(k.transpose(0, 2, 1))

    nc = _get_program()
    in_maps = [
        {
            "qT": qT[c * HPC:(c + 1) * HPC],
            "kT": kT[c * HPC:(c + 1) * HPC],
            "v": v[c * HPC:(c + 1) * HPC],
        }
        for c in range(NCORES)
    ]
    res = run_bass_kernel_spmd(nc, in_maps, list(range(NCORES)))
    kernel._last = res
    out = np.concatenate([res.results[c]["out"] for c in range(NCORES)], axis=0)
    return out.reshape(B, H, T, D)


# revision 17
# speedup vs baseline: 54.5748x; 54.5748x over previous
"""Causal multi-head attention (B=2, H=12, T=2048, D=64) on 8 Trainium2 NeuronCores.

Sharding: the 24 (batch, head) pairs are split 3-per-core across 8 cores.
Per head the device kernel computes, in transposed-score layout:

    S^T[kv, q] = K @ Q^T            (PE, fp16 in / fp32 PSUM out)
    P^T        = exp(S^T * 1/8)     (ACT, reads PSUM, writes fp16 SBUF;
                                     no max-subtraction: |scores| <= ~6)
    diag block masked in-place on GPSIMD (affine_select, upper-triangular keep)
    O'^T[65, q] = [V | ones] PV matmul (PE), so row 64 of O'^T accumulates
                  the softmax denominators
    O = (O'^T[0:64] / O'^T[64]).T   (PE transpose + DVE recip/scale)

The 136 causal (q-tile, kv-block) score blocks of each head form one linear
stream, chunked into PSUM megas of `mega_blks` blocks; exp is issued once per
mega. The chunk stream is software-pipelined one chunk ahead of the PV
consumers and runs continuously across the 3 heads.

`repeat` > 1 wraps the whole body in a hardware For_i loop — a timing aid
(run the same computation N times in one NEFF), not used in production.

Self-contained: only imports numpy + the installed concourse/bass stack.
"""

import os
import numpy as np

B, H, T, D = 2, 12, 2048, 64
NCORES = 8
HPC = (B * H) // NCORES      # heads per core = 3
NQT = T // 128               # 16 q tiles of 128 rows
MEGA_BLKS = 8                # kv blocks per PSUM score tile (8*128 cols = 2 banks)
SCALE = 1.0 / 8.0            # 1/sqrt(D)

_cache = {}


def build_program(
    mega_blks=MEGA_BLKS,
    smega_bufs=3,
    pt_bufs=3,
    oq_bufs=1,
    io_bufs=2,
    repeat=1,
    ablate=(),
):
    import concourse.bacc as bacc
    import concourse.mybir as mybir
    import concourse.tile as tile
    from concourse.masks import make_upper_triangular, make_identity

    f16 = mybir.dt.float16
    f32 = mybir.dt.float32
    Exp = mybir.ActivationFunctionType.Exp

    nc = bacc.Bacc(None)
    qT_d = nc.dram_tensor("qT", [HPC, D, T], f16, kind="ExternalInput")
    kT_d = nc.dram_tensor("kT", [HPC, D, T], f16, kind="ExternalInput")
    v_d = nc.dram_tensor("v", [HPC, T, D], f16, kind="ExternalInput")
    o_d = nc.dram_tensor("out", [HPC, T, D], f32, kind="ExternalOutput")

    blocks = [(i, j) for i in range(NQT) for j in range(i + 1)]
    nblk = len(blocks)                      # 136
    off = {bl: 128 * n for n, bl in enumerate(blocks)}
    chunks = [blocks[c0:c0 + mega_blks] for c0 in range(0, nblk, mega_blks)]
    # q-tile i of a head is fully scored once block (i, i) has been emitted
    fin_chunk = {}
    for ci, ch in enumerate(chunks):
        for (i, j) in ch:
            if i == j:
                fin_chunk.setdefault(ci, []).append(i)

    with tile.TileContext(nc) as tc:
        with (
            tc.tile_pool(name="consts", bufs=1) as consts,
            tc.tile_pool(name="qk", bufs=io_bufs) as qk,
            tc.tile_pool(name="vpool", bufs=io_bufs) as vpool,
            tc.tile_pool(name="ptpool", bufs=pt_bufs) as ptpool,
            tc.tile_pool(name="odrain", bufs=2) as odrain,
            tc.tile_pool(name="otsb", bufs=2) as otsb,
            tc.tile_pool(name="rp", bufs=2) as rp,
            tc.tile_pool(name="onorm", bufs=2) as onorm,
            tc.tile_pool(name="smega", bufs=smega_bufs, space="PSUM") as smega,
            tc.tile_pool(name="oqp", bufs=oq_bufs, space="PSUM") as oqp,
            tc.tile_pool(name="otp", bufs=1, space="PSUM") as otp,
        ):
            U = consts.tile([128, 128], f16)
            make_upper_triangular(nc, U[:], val=1.0, diag=True)
            ident = consts.tile([128, 128], f32)
            make_identity(nc, ident[:])
            # Warm the ACT exp table set while the first DMAs are in flight.
            warm = consts.tile([128, 1], f32)
            nc.scalar.activation(warm[:], U[:, 0:1], Exp)

            def emit_body():
                heads = {}

                def emit_loads(h):
                    qt = qk.tile([D, T], f16, tag="qt")
                    kt = qk.tile([D, T], f16, tag="kt")
                    for c in range(0, T, 512):
                        nc.sync.dma_start(kt[:, c:c + 512], kT_d[h, :, c:c + 512])
                        nc.sync.dma_start(qt[:, c:c + 512], qT_d[h, :, c:c + 512])
                    vp = vpool.tile([128, NQT, D + 1], f16)
                    nc.sync.dma_start(
                        vp[:, :, 0:D], v_d[h].rearrange("(j p) d -> p j d", p=128)
                    )
                    nc.vector.memset(vp[:, :, D], 1.0)
                    pt = ptpool.tile([128, nblk * 128], f16, tag="pt")
                    heads[h] = {"qt": qt, "kt": kt, "vp": vp, "pt": pt}

                def emit_chunk(h, ci):
                    hd = heads[h]
                    ch = chunks[ci]
                    sm = smega.tile([128, len(ch) * 128], f32, tag="sm")
                    for idx, (i, j) in enumerate(ch):
                        nc.tensor.matmul(
                            sm[:, idx * 128:(idx + 1) * 128],
                            hd["kt"][:, j * 128:(j + 1) * 128],
                            hd["qt"][:, i * 128:(i + 1) * 128],
                        )
                    nc.scalar.activation(
                        hd["pt"][:, off[ch[0]]:off[ch[-1]] + 128],
                        sm[:],
                        Exp,
                        scale=SCALE,
                    )

                def emit_drain(h, oq_tile, i0):
                    if "tail" in ablate:
                        return None
                    od = odrain.tile([D + 1, 512], f32)
                    nc.vector.tensor_copy(od[:], oq_tile[:])
                    return (h, od, i0)

                def emit_rest(h, od, i0):
                    ot = otp.tile([128, 4, D + 1], f32)
                    for t in range(4):
                        nc.tensor.transpose(
                            ot[:, t, :],
                            od[:, t * 128:(t + 1) * 128],
                            ident[0:D + 1, 0:D + 1],
                        )
                    ots = otsb.tile([128, 4, D + 1], f32)
                    nc.vector.tensor_copy(ots[:], ot[:])
                    r = rp.tile([128, 4], f32)
                    nc.vector.reciprocal(r[:], ots[:, :, D])
                    on = onorm.tile([128, 4, D], f32)
                    for t in range(4):
                        nc.vector.tensor_scalar_mul(
                            on[:, t, :], ots[:, t, 0:D], r[:, t:t + 1]
                        )
                    nc.sync.dma_start(
                        o_d[h, i0 * 128:(i0 + 4) * 128, :].rearrange(
                            "(t p) d -> p t d", p=128
                        ),
                        on[:],
                    )

                state = {"oq": None, "pend": None, "pend2": None}

                def emit_pv(h, i):
                    hd = heads[h]
                    nb = i + 1
                    if i % 4 == 0:
                        if state["pend2"] is not None:
                            emit_rest(*state["pend2"])
                            state["pend2"] = None
                        if state["pend"] is not None:
                            state["pend2"] = emit_drain(*state["pend"])
                            state["pend"] = None
                        oq = oqp.tile([D + 1, 512], f32, tag="oq")
                        state["oq"] = oq
                    pt = hd["pt"]
                    pd = pt[:, off[(i, i)]:off[(i, i)] + 128]
                    if "mask" not in ablate:
                        nc.gpsimd.affine_select(
                            out=pd, in_=pd,
                            compare_op=mybir.AluOpType.is_ge,
                            fill=0.0, base=0,
                            # keep where (y - x) >= 0, i.e. q >= kv
                            pattern=[[1, 128]], channel_multiplier=-1,
                        )
                    osl = state["oq"][:, (i % 4) * 128:(i % 4 + 1) * 128]
                    for j in range(nb if "pv" not in ablate else 1):
                        nc.tensor.matmul(
                            osl,
                            hd["vp"][:, j, :],
                            pt[:, off[(i, j)]:off[(i, j)] + 128],
                            start=(j == 0),
                            stop=(j == nb - 1),
                        )
                    if i % 4 == 3:
                        state["pend"] = (h, state["oq"], i - 3)

                # one software-pipelined stream over (head, chunk)
                nch = len(chunks)
                stream = [(h, ci) for h in range(HPC) for ci in range(nch)]
                npre = min(io_bufs, HPC)
                for h in range(npre):
                    emit_loads(h)
                for n, (h, ci) in enumerate(stream):
                    if n == 0:
                        emit_chunk(h, ci)
                    if n + 1 < len(stream):
                        h2, ci2 = stream[n + 1]
                        if ci2 == 0 and h2 >= npre:
                            emit_loads(h2)
                        emit_chunk(h2, ci2)
                    for i in fin_chunk.get(ci, []):
                        emit_pv(h, i)
                if state["pend2"] is not None:
                    emit_rest(*state["pend2"])
                if state["pend"] is not None:
                    p2 = emit_drain(*state["pend"])
                    if p2 is not None:
                        emit_rest(*p2)

            if repeat == 1:
                emit_body()
            else:
                import concourse.mybir as _mb
                engs = (
                    _mb.EngineType.PE,
                    _mb.EngineType.Activation,
                    _mb.EngineType.DVE,
                    _mb.EngineType.SP,
                    _mb.EngineType.Pool,
                )
                with tc.For_i(0, repeat, 1, hint_engines=engs):
                    emit_body()

    nc.compile()
    return nc


def _get_program():
    if "nc" not in _cache:
        os.environ.setdefault("MYCRO_LOCAL_CACHE", "1")
        _cache["nc"] = build_program()
    return _cache["nc"]


def kernel(q, k, v):
    from concourse.bass_utils import run_bass_kernel_spmd

    q = np.asarray(q).reshape(B * H, T, D).astype(np.float16)
    k = np.asarray(k).reshape(B * H, T, D).astype(np.float16)
    v = np.ascontiguousarray(np.asarray(v).reshape(B * H, T, D).astype(np.float16))
    qT = np.ascontiguousarray(q.transpose(0, 2, 1))
    kT = np.ascontiguousarray(k.transpose(0, 2, 1))

    nc = _get_program()
    in_maps = [
        {
            "qT": qT[c * HPC:(c + 1) * HPC],
            "kT": kT[c * HPC:(c + 1) * HPC],
            "v": v[c * HPC:(c + 1) * HPC],
        }
        for c in range(NCORES)
    ]
    res = run_bass_kernel_spmd(nc, in_maps, list(range(NCORES)))
    kernel._last = res
    out = np.concatenate([res.results[c]["out"] for c in range(NCORES)], axis=0)
    return out.reshape(B, H, T, D)
